# revision 30
# baseline (speedup 1.0000x reference)
"""Device-pure Fourier-domain kernel for nn_EquiLinearRegToReg, v4.

Block-circulant over k: DFT diagonalization, three on-device stages.
The two partition-relayouts (S1->S2, S2->S3) bounce through DRAM
scratch (SBUF-side DMA APs only support one partition dim, so a
direct SBUF->SBUF exchange is not expressible). v4: bf16, bf16
output (upcast on host), deduplicated weight slabs (im-planes reuse
the re-planes' Wr via stationary APs: ship {Wr, Wi, -Wi} = 2.9MB),
input loads ahead of weight loads, S2/S3 interleave, DMA queues
assigned to avoid head-of-line blocking.

S1: f_hat = DFT_x(field): 32 matmuls, block-diag DFT stationary,
    partitions (x,i8) -> (i8,plane); scatter/gather to (io,i127).
S2: per-frequency complex matmuls (K=i127, N=bp=512), 120 matmuls,
    output partitions j127; scatter/gather to (j8,plane).
S3: iDFT, partitions (j8,plane) -> (j8,y), 32 matmuls, bf16 out.

Plane order: [w0, re1, im1, ..., re7, im7, w8] (16 real planes).
"""

import os
import numpy as np
import ml_dtypes

import concourse.mybir as mybir
import concourse.tile as tile
from concourse import bacc
from concourse.bass_utils import run_bass_kernel_spmd

BATCH, NUM_PART, IN_FEAT, OUT_FEAT, K = 8, 512, 256, 256, 16
N_CORES = 8
P = 128
IO = IN_FEAT // P
NIG = IN_FEAT // 8          # 32 i-groups of 8
NJG = OUT_FEAT // 8         # 32 j-groups of 8
JC = OUT_FEAT // P          # 2 j-chunks of 128

BF16 = ml_dtypes.bfloat16

_CACHE = {}

PLANES = [(0, "re")] + [(w, k) for w in range(1, 8) for k in ("re", "im")] + [(8, "re")]

# per-w slab table: w=0,8 ship [Wr0, Wr1]; w=1..7 ship
# [Wr0, Wr1, Wi0, Wi1, -Wi0, -Wi1] (6 slabs); im-planes reuse Wr.
NSLAB = {w: (2 if w in (0, 8) else 6) for w in range(9)}


def _s2_slabs(pp):
    """For out-plane pp: list of (w, slab_idx, io) stationary slabs, in
    accumulation order (io-major pairs)."""
    w, kind = PLANES[pp]
    if w in (0, 8):
        return [(w, 0, 0), (w, 1, 1)]
    if kind == "re":   # Hr = Fr Wr + Fi (-Wi)
        return [(w, 0, 0), (w, 4, 0), (w, 1, 1), (w, 5, 1)]
    else:              # Hi = Fr Wi + Fi Wr
        return [(w, 2, 0), (w, 0, 0), (w, 3, 1), (w, 1, 1)]


def _s2_planes(pp):
    """Moving-tensor plane q for each slab of _s2_slabs(pp)."""
    w, kind = PLANES[pp]
    if w in (0, 8):
        return [pp, pp]
    if kind == "re":
        return [pp, pp + 1, pp, pp + 1]
    else:
        return [pp - 1, pp, pp - 1, pp]


def _cf():
    C = np.zeros((K, K))
    x = np.arange(K)
    for p, (w, kind) in enumerate(PLANES):
        C[:, p] = np.cos(2 * np.pi * w * x / K) if kind == "re" else -np.sin(2 * np.pi * w * x / K)
    return C


def _ci():
    C = np.zeros((K, K))
    y = np.arange(K)
    for p, (w, kind) in enumerate(PLANES):
        s = 1.0 / K if w in (0, 8) else 2.0 / K
        C[p, :] = s * np.cos(2 * np.pi * w * y / K) if kind == "re" else -s * np.sin(2 * np.pi * w * y / K)
    return C


def _build():
    if "nc" in _CACHE:
        return _CACHE["nc"]
    f32 = mybir.dt.float32
    b16 = mybir.dt.bfloat16

    nc = bacc.Bacc(None, target_bir_lowering=False, debug=False)
    fieldx_d = nc.dram_tensor("fieldx", [NIG // 4, P, 4, NUM_PART], b16, kind="ExternalInput")
    b1_d = nc.dram_tensor("b1", [P, P], b16, kind="ExternalInput")
    b3_d = nc.dram_tensor("b3", [P, P], b16, kind="ExternalInput")
    w2_ds = [nc.dram_tensor(f"w2_{w}", [P, NSLAB[w], OUT_FEAT], b16, kind="ExternalInput")
             for w in range(9)]
    # scratch, laid out so every scatter/gather is one large affine DMA
    fh_ds = [nc.dram_tensor(f"fh{h}", [NIG // 2, P, NUM_PART], b16) for h in range(2)]
    oh_ds = [nc.dram_tensor(f"oh{h}", [P, K, NUM_PART], b16) for h in range(2)]
    out_d = nc.dram_tensor("out", [NJG, P, NUM_PART], b16, kind="ExternalOutput")

    with tile.TileContext(nc) as tc:
        with (
            tc.tile_pool(name="const", bufs=1) as const,
            tc.tile_pool(name="sb", bufs=8) as sb,
            tc.tile_pool(name="st", bufs=8) as st,
            tc.tile_pool(name="psum", bufs=4, space="PSUM") as psum,
        ):
            b1 = const.tile([P, P], b16, name="b1", tag="b1", bufs=1)
            b3 = const.tile([P, P], b16, name="b3", tag="b3", bufs=1)
            # per-w weight tiles so S2 only waits on its own slab's load
            w2w = [const.tile([P, NSLAB[w], OUT_FEAT], b16, name=f"w2w{w}",
                              tag=f"w2w{w}", bufs=1) for w in range(9)]
            fht = const.tile([P, IO, K, NUM_PART], b16, name="fht", tag="fh", bufs=1)

            # inputs first: fx, then weights (weights are consumed by S2
            # which cannot start before the f_hat bounce completes anyway)
            nc.sync.dma_start(b1[:], b1_d[:])
            nc.sync.dma_start(b3[:], b3_d[:])
            fxs = []
            for b in range(NIG // 4):
                fx = sb.tile([P, 4, NUM_PART], b16, tag="fx", name=f"fx{b}")
                nc.sync.dma_start(fx[:], fieldx_d[b])
                fxs.append(fx)

            def evict(dst, src, eng):
                if eng == "v":
                    nc.vector.tensor_copy(dst, src)
                elif eng == "s":
                    nc.scalar.copy(dst, src)
                else:
                    nc.gpsimd.tensor_copy(dst, src)

            # ---- S1 ----  (scatters on scalar, gathers on sync)
            fhvs = [
                fh_ds[h][:].rearrange("ig r bp -> (ig r) bp")
                .rearrange("(ig r) bp -> r ig bp", r=P)
                for h in range(2)
            ]
            for b in range(NIG // 4):
                sg = st.tile([P, 4, NUM_PART], b16, tag="sg", bufs=6, name=f"sg{b}")
                for half in range(2):
                    acc = psum.tile([P, 2, NUM_PART], f32, tag="ps",
                                    name=f"s1p{b}_{half}")
                    for q2 in range(2):
                        nc.tensor.matmul(acc[:, q2, :], b1[:],
                                         fxs[b][:, half * 2 + q2, :],
                                         start=True, stop=True)
                    evict(sg[:, half * 2:half * 2 + 2, :], acc[:],
                          "v" if half == 0 else "s")
                h, bh = divmod(b, 4)
                nc.gpsimd.dma_start(fhvs[h][:, bh * 4:(bh + 1) * 4, :], sg[:])

                # gather this block's 32-partition slice immediately:
                # each fht partition slice depends on exactly one scatter,
                # so gathers chase scatters block-by-block; halves go to
                # different rings to drain in parallel
                fh_flat = fh_ds[h][:].rearrange("ig r bp -> (ig r) bp")
                fh_iq = fh_flat.rearrange("(i q) bp -> i q bp", q=K)
                eng = nc.sync if h == 0 else nc.scalar
                eng.dma_start(
                    fht[bh * 32:(bh + 1) * 32, h],
                    fh_iq[bh * 32:(bh + 1) * 32, :, :])

            # weight loads behind the gathers on the sync ring
            for w in range(9):
                nc.sync.dma_start(w2w[w][:], w2_ds[w][:])

            # ---- S2 / S3 interleaved ----
            ohvs = [
                oh_ds[jc][:].rearrange("j p bp -> (j p) bp")
                .rearrange("(jg r) bp -> r jg bp", r=P)
                for jc in range(JC)
            ]

            def s2_pair(jc, ppp):
                hg = st.tile([P, 2, NUM_PART], b16, tag="hg", bufs=4,
                             name=f"hg{jc}_{ppp}")
                acc = psum.tile([P, 2, NUM_PART], f32, tag="ps",
                                name=f"s2p{jc}_{ppp}")
                for q2 in range(2):
                    pp = ppp * 2 + q2
                    slabs = _s2_slabs(pp)
                    qs = _s2_planes(pp)
                    for ki, ((w, si, io_), q) in enumerate(zip(slabs, qs)):
                        nc.tensor.matmul(
                            acc[:, q2, :],
                            w2w[w][:, si, jc * P:(jc + 1) * P],
                            fht[:, io_, q, :],
                            start=(ki == 0),
                            stop=(ki == len(slabs) - 1),
                        )
                evict(hg[:], acc[:], "v" if ppp % 2 == 0 else "s")
                nc.scalar.dma_start(oh_ds[jc][:, ppp * 2:ppp * 2 + 2, :], hg[:])

            def s3_quad(bb):
                # bb in 0..3: gather 8 j-groups, 4 paired matmuls + stores
                jc, base = divmod(bb * 8, NJG // 2)
                oht = sb.tile([P, 8, NUM_PART], b16, tag="oht", bufs=3,
                              name=f"oht{bb}")
                nc.sync.dma_start(oht[:], ohvs[jc][:, base:base + 8, :])
                for jp in range(4):
                    og = st.tile([P, 2, NUM_PART], b16, tag="og", bufs=4,
                                 name=f"og{bb}_{jp}")
                    acc = psum.tile([P, 2, NUM_PART], f32, tag="ps",
                                    name=f"s3p{bb}_{jp}")
                    for r in range(2):
                        nc.tensor.matmul(acc[:, r, :], b3[:],
                                         oht[:, jp * 2 + r, :],
                                         start=True, stop=True)
                    evict(og[:], acc[:], "v" if jp % 2 == 0 else "s")
                    jg = bb * 8 + jp * 2
                    nc.gpsimd.dma_start(
                        out_d[jg:jg + 2].rearrange("g p bp -> p g bp"), og[:])

            def s3_pairfine(pb):
                # last quad, pair-granularity to shorten the drain tail
                jc, base = divmod(pb * 2, NJG // 2)
                oht = sb.tile([P, 2, NUM_PART], b16, tag="oht2", bufs=4,
                              name=f"ohtp{pb}")
                nc.sync.dma_start(oht[:], ohvs[jc][:, base:base + 2, :])
                og = st.tile([P, 2, NUM_PART], b16, tag="og", bufs=4,
                             name=f"ogp{pb}")
                acc = psum.tile([P, 2, NUM_PART], f32, tag="ps",
                                name=f"s3pp{pb}")
                for r in range(2):
                    nc.tensor.matmul(acc[:, r, :], b3[:], oht[:, r, :],
                                     start=True, stop=True)
                evict(og[:], acc[:], "v" if pb % 2 == 0 else "s")
                jg = pb * 2
                nc.gpsimd.dma_start(
                    out_d[jg:jg + 2].rearrange("g p bp -> p g bp"), og[:])

            for ppp in range(8):
                s2_pair(0, ppp)
            for i in range(8):
                s2_pair(1, i)
                if i % 4 == 3:
                    s3_quad(i // 4)
            s3_quad(2)
            for pb in range(12, 16):
                s3_pairfine(pb)

    nc.compile()
    _CACHE["nc"] = nc
    return nc


def _prep_inputs(field_feat, weights):
    field_feat = np.ascontiguousarray(field_feat, dtype=np.float32)
    weights = np.ascontiguousarray(weights, dtype=np.float32)

    Cf, Ci = _cf(), _ci()
    B1 = np.zeros((P, P), np.float32)
    for x in range(K):
        for i8 in range(8):
            B1[x * 8 + i8, i8 * 16:(i8 + 1) * 16] = Cf[x]
    B3 = np.zeros((P, P), np.float32)
    for j8 in range(8):
        B3[j8 * 16:(j8 + 1) * 16, j8 * 16:(j8 + 1) * 16] = Ci
    Wf = np.fft.fft(weights, axis=2)

    # per-w slab stacks: [Wr0, Wr1] (+ [Wi0, Wi1, -Wi0, -Wi1] for w=1..7)
    # where suffix = io half (rows io*128..io*128+127)
    w2s = {}
    for w in range(9):
        Wr = Wf[:, :, w].real.astype(np.float32)
        Wi = Wf[:, :, w].imag.astype(np.float32)
        slabs = [Wr[:P], Wr[P:]]
        if w not in (0, 8):
            slabs += [Wi[:P], Wi[P:], -Wi[:P], -Wi[P:]]
        w2s[f"w2_{w}"] = np.ascontiguousarray(
            np.stack(slabs, axis=1).astype(BF16))       # [P, nslab, OUT]

    in_maps = []
    b1 = B1.astype(BF16)
    b3 = B3.astype(BF16)
    for c in range(N_CORES):
        f = field_feat[c].transpose(1, 2, 0)                  # [i, x, bp]
        fx = f.reshape(NIG, 8, K, NUM_PART).transpose(0, 2, 1, 3)
        fx = fx.reshape(NIG // 4, 4, P, NUM_PART).transpose(0, 2, 1, 3)
        fx = np.ascontiguousarray(fx.astype(BF16))
        in_maps.append({"fieldx": fx, "b1": b1, "b3": b3, **w2s})
    return in_maps


def kernel(field_feat, weights):
    nc = _build()
    in_maps = _prep_inputs(field_feat, weights)
    trace = bool(int(os.environ.get("KERNEL_TRACE", "0")))
    # NRT occasionally reports a transient EXEC_UNIT_UNRECOVERABLE on the
    # first execute of a fresh session; a retry on a new session passes.
    for attempt in range(3):
        try:
            res = run_bass_kernel_spmd(nc, in_maps, list(range(N_CORES)),
                                       trace=trace)
            break
        except Exception:  # noqa: BLE001
            if attempt == 2:
                raise
    if trace:
        kernel.last_exec_time_ns = res.exec_time_ns
        kernel.last_results = res
    # out[jg, j8*16+y, bp] -> [bp, j, y]
    outs = []
    for c in range(N_CORES):
        o = np.asarray(res.results[c]["out"]).reshape(NJG, 8, K, NUM_PART)
        outs.append(o.transpose(3, 0, 1, 2).reshape(NUM_PART, OUT_FEAT, K))
    return np.stack(outs).reshape(BATCH, NUM_PART, OUT_FEAT, K).astype(np.float32)


# revision 31
# speedup vs baseline: 1.0788x; 1.0788x over previous
"""Device-pure Fourier-domain kernel for nn_EquiLinearRegToReg, v4.

Block-circulant over k: DFT diagonalization, three on-device stages.
The two partition-relayouts (S1->S2, S2->S3) bounce through DRAM
scratch (SBUF-side DMA APs only support one partition dim, so a
direct SBUF->SBUF exchange is not expressible). v4: bf16, bf16
output (upcast on host), deduplicated weight slabs (im-planes reuse
the re-planes' Wr via stationary APs: ship {Wr, Wi, -Wi} = 2.9MB),
input loads ahead of weight loads, S2/S3 interleave, DMA queues
assigned to avoid head-of-line blocking.

S1: f_hat = DFT_x(field): 32 matmuls, block-diag DFT stationary,
    partitions (x,i8) -> (i8,plane); scatter/gather to (io,i127).
S2: per-frequency complex matmuls (K=i127, N=bp=512), 120 matmuls,
    output partitions j127; scatter/gather to (j8,plane).
S3: iDFT, partitions (j8,plane) -> (j8,y), 32 matmuls, bf16 out.

Plane order: [w0, re1, im1, ..., re7, im7, w8] (16 real planes).
"""

import os
import numpy as np
import ml_dtypes

import concourse.mybir as mybir
import concourse.tile as tile
from concourse import bacc
from concourse.bass_utils import run_bass_kernel_spmd

BATCH, NUM_PART, IN_FEAT, OUT_FEAT, K = 8, 512, 256, 256, 16
N_CORES = 8
P = 128
IO = IN_FEAT // P
NIG = IN_FEAT // 8          # 32 i-groups of 8
NJG = OUT_FEAT // 8         # 32 j-groups of 8
JC = OUT_FEAT // P          # 2 j-chunks of 128

BF16 = ml_dtypes.bfloat16

_CACHE = {}

PLANES = [(0, "re")] + [(w, k) for w in range(1, 8) for k in ("re", "im")] + [(8, "re")]

# per-w slab table: w=0,8 ship [Wr0, Wr1]; w=1..7 ship
# [Wr0, Wr1, Wi0, Wi1, -Wi0, -Wi1] (6 slabs); im-planes reuse Wr.
NSLAB = {w: (2 if w in (0, 8) else 6) for w in range(9)}


def _s2_slabs(pp):
    """For out-plane pp: list of (w, slab_idx, io) stationary slabs, in
    accumulation order (io-major pairs)."""
    w, kind = PLANES[pp]
    if w in (0, 8):
        return [(w, 0, 0), (w, 1, 1)]
    if kind == "re":   # Hr = Fr Wr + Fi (-Wi)
        return [(w, 0, 0), (w, 4, 0), (w, 1, 1), (w, 5, 1)]
    else:              # Hi = Fr Wi + Fi Wr
        return [(w, 2, 0), (w, 0, 0), (w, 3, 1), (w, 1, 1)]


def _s2_planes(pp):
    """Moving-tensor plane q for each slab of _s2_slabs(pp)."""
    w, kind = PLANES[pp]
    if w in (0, 8):
        return [pp, pp]
    if kind == "re":
        return [pp, pp + 1, pp, pp + 1]
    else:
        return [pp - 1, pp, pp - 1, pp]


def _cf():
    C = np.zeros((K, K))
    x = np.arange(K)
    for p, (w, kind) in enumerate(PLANES):
        C[:, p] = np.cos(2 * np.pi * w * x / K) if kind == "re" else -np.sin(2 * np.pi * w * x / K)
    return C


def _ci():
    C = np.zeros((K, K))
    y = np.arange(K)
    for p, (w, kind) in enumerate(PLANES):
        s = 1.0 / K if w in (0, 8) else 2.0 / K
        C[p, :] = s * np.cos(2 * np.pi * w * y / K) if kind == "re" else -s * np.sin(2 * np.pi * w * y / K)
    return C


def _build():
    if "nc" in _CACHE:
        return _CACHE["nc"]
    f32 = mybir.dt.float32
    b16 = mybir.dt.bfloat16

    nc = bacc.Bacc(None, target_bir_lowering=False, debug=False)
    fieldx_d = nc.dram_tensor("fieldx", [NIG // 4, P, 4, NUM_PART], b16, kind="ExternalInput")
    b1_d = nc.dram_tensor("b1", [P, P], b16, kind="ExternalInput")
    b3_d = nc.dram_tensor("b3", [P, P], b16, kind="ExternalInput")
    w2_ds = [nc.dram_tensor(f"w2_{w}", [P, NSLAB[w], OUT_FEAT], b16, kind="ExternalInput")
             for w in range(9)]
    # scratch, laid out so every scatter/gather is one large affine DMA
    fh_ds = [nc.dram_tensor(f"fh{h}", [NIG // 2, P, NUM_PART], b16) for h in range(2)]
    oh_ds = [nc.dram_tensor(f"oh{h}", [P, K, NUM_PART], b16) for h in range(2)]
    out_d = nc.dram_tensor("out", [NJG, P, NUM_PART], b16, kind="ExternalOutput")

    with tile.TileContext(nc) as tc:
        with (
            tc.tile_pool(name="const", bufs=1) as const,
            tc.tile_pool(name="sb", bufs=8) as sb,
            tc.tile_pool(name="st", bufs=8) as st,
            tc.tile_pool(name="psum", bufs=4, space="PSUM") as psum,
        ):
            b1 = const.tile([P, P], b16, name="b1", tag="b1", bufs=1)
            b3 = const.tile([P, P], b16, name="b3", tag="b3", bufs=1)
            # per-w weight tiles so S2 only waits on its own slab's load
            w2w = [const.tile([P, NSLAB[w], OUT_FEAT], b16, name=f"w2w{w}",
                              tag=f"w2w{w}", bufs=1) for w in range(9)]
            fht = const.tile([P, IO, K, NUM_PART], b16, name="fht", tag="fh", bufs=1)

            # inputs first: fx, then weights (weights are consumed by S2
            # which cannot start before the f_hat bounce completes anyway)
            nc.sync.dma_start(b1[:], b1_d[:])
            nc.sync.dma_start(b3[:], b3_d[:])
            fxs = []
            for b in range(NIG // 4):
                fx = sb.tile([P, 4, NUM_PART], b16, tag="fx", name=f"fx{b}")
                nc.sync.dma_start(fx[:], fieldx_d[b])
                fxs.append(fx)

            def evict(dst, src, eng):
                if eng == "v":
                    nc.vector.tensor_copy(dst, src)
                elif eng == "s":
                    nc.scalar.copy(dst, src)
                else:
                    nc.gpsimd.tensor_copy(dst, src)

            # ---- S1 ----  (scatters on scalar, gathers on sync)
            fhvs = [
                fh_ds[h][:].rearrange("ig r bp -> (ig r) bp")
                .rearrange("(ig r) bp -> r ig bp", r=P)
                for h in range(2)
            ]
            for b in range(NIG // 4):
                sg = st.tile([P, 4, NUM_PART], b16, tag="sg", bufs=6, name=f"sg{b}")
                for half in range(2):
                    acc = psum.tile([P, 2, NUM_PART], f32, tag="ps",
                                    name=f"s1p{b}_{half}")
                    for q2 in range(2):
                        nc.tensor.matmul(acc[:, q2, :], b1[:],
                                         fxs[b][:, half * 2 + q2, :],
                                         start=True, stop=True)
                    evict(sg[:, half * 2:half * 2 + 2, :], acc[:],
                          "v" if half == 0 else "s")
                h, bh = divmod(b, 4)
                nc.gpsimd.dma_start(fhvs[h][:, bh * 4:(bh + 1) * 4, :], sg[:])

                # gather half h as soon as its 4 scatters are in flight;
                # the two halves go to different rings so they drain in
                # parallel instead of serializing on one FIFO
                if bh == 3:
                    fh_flat = fh_ds[h][:].rearrange("ig r bp -> (ig r) bp")
                    fh_iq = fh_flat.rearrange("(i q) bp -> i q bp", q=K)
                    eng = nc.sync if h == 0 else nc.scalar
                    for qh in range(2):
                        eng.dma_start(
                            fht[:, h, qh * 8:(qh + 1) * 8, :],
                            fh_iq[:, qh * 8:(qh + 1) * 8, :])

            # weight loads behind the gathers on the sync ring
            for w in range(9):
                nc.sync.dma_start(w2w[w][:], w2_ds[w][:])

            # ---- S2 / S3 interleaved ----
            ohvs = [
                oh_ds[jc][:].rearrange("j p bp -> (j p) bp")
                .rearrange("(jg r) bp -> r jg bp", r=P)
                for jc in range(JC)
            ]

            def s2_pair(jc, ppp):
                hg = st.tile([P, 2, NUM_PART], b16, tag="hg", bufs=4,
                             name=f"hg{jc}_{ppp}")
                acc = psum.tile([P, 2, NUM_PART], f32, tag="ps",
                                name=f"s2p{jc}_{ppp}")
                for q2 in range(2):
                    pp = ppp * 2 + q2
                    slabs = _s2_slabs(pp)
                    qs = _s2_planes(pp)
                    for ki, ((w, si, io_), q) in enumerate(zip(slabs, qs)):
                        nc.tensor.matmul(
                            acc[:, q2, :],
                            w2w[w][:, si, jc * P:(jc + 1) * P],
                            fht[:, io_, q, :],
                            start=(ki == 0),
                            stop=(ki == len(slabs) - 1),
                        )
                evict(hg[:], acc[:], "v" if ppp % 2 == 0 else "s")
                nc.scalar.dma_start(oh_ds[jc][:, ppp * 2:ppp * 2 + 2, :], hg[:])

            def s3_quad(bb):
                # bb in 0..3: gather 8 j-groups, 4 paired matmuls + stores
                jc, base = divmod(bb * 8, NJG // 2)
                oht = sb.tile([P, 8, NUM_PART], b16, tag="oht", bufs=3,
                              name=f"oht{bb}")
                nc.sync.dma_start(oht[:], ohvs[jc][:, base:base + 8, :])
                for jp in range(4):
                    og = st.tile([P, 2, NUM_PART], b16, tag="og", bufs=4,
                                 name=f"og{bb}_{jp}")
                    acc = psum.tile([P, 2, NUM_PART], f32, tag="ps",
                                    name=f"s3p{bb}_{jp}")
                    for r in range(2):
                        nc.tensor.matmul(acc[:, r, :], b3[:],
                                         oht[:, jp * 2 + r, :],
                                         start=True, stop=True)
                    evict(og[:], acc[:], "v" if jp % 2 == 0 else "s")
                    jg = bb * 8 + jp * 2
                    nc.gpsimd.dma_start(
                        out_d[jg:jg + 2].rearrange("g p bp -> p g bp"), og[:])

            def s3_pairfine(pb):
                # last quad, pair-granularity to shorten the drain tail
                jc, base = divmod(pb * 2, NJG // 2)
                oht = sb.tile([P, 2, NUM_PART], b16, tag="oht2", bufs=4,
                              name=f"ohtp{pb}")
                nc.sync.dma_start(oht[:], ohvs[jc][:, base:base + 2, :])
                og = st.tile([P, 2, NUM_PART], b16, tag="og", bufs=4,
                             name=f"ogp{pb}")
                acc = psum.tile([P, 2, NUM_PART], f32, tag="ps",
                                name=f"s3pp{pb}")
                for r in range(2):
                    nc.tensor.matmul(acc[:, r, :], b3[:], oht[:, r, :],
                                     start=True, stop=True)
                evict(og[:], acc[:], "v" if pb % 2 == 0 else "s")
                jg = pb * 2
                nc.gpsimd.dma_start(
                    out_d[jg:jg + 2].rearrange("g p bp -> p g bp"), og[:])

            for ppp in range(8):
                s2_pair(0, ppp)
            for i in range(8):
                s2_pair(1, i)
                if i % 4 == 3:
                    s3_quad(i // 4)
            s3_quad(2)
            for pb in range(12, 16):
                s3_pairfine(pb)

    nc.compile()
    _CACHE["nc"] = nc
    return nc


def _prep_inputs(field_feat, weights):
    field_feat = np.ascontiguousarray(field_feat, dtype=np.float32)
    weights = np.ascontiguousarray(weights, dtype=np.float32)

    Cf, Ci = _cf(), _ci()
    B1 = np.zeros((P, P), np.float32)
    for x in range(K):
        for i8 in range(8):
            B1[x * 8 + i8, i8 * 16:(i8 + 1) * 16] = Cf[x]
    B3 = np.zeros((P, P), np.float32)
    for j8 in range(8):
        B3[j8 * 16:(j8 + 1) * 16, j8 * 16:(j8 + 1) * 16] = Ci
    Wf = np.fft.fft(weights, axis=2)

    # per-w slab stacks: [Wr0, Wr1] (+ [Wi0, Wi1, -Wi0, -Wi1] for w=1..7)
    # where suffix = io half (rows io*128..io*128+127)
    w2s = {}
    for w in range(9):
        Wr = Wf[:, :, w].real.astype(np.float32)
        Wi = Wf[:, :, w].imag.astype(np.float32)
        slabs = [Wr[:P], Wr[P:]]
        if w not in (0, 8):
            slabs += [Wi[:P], Wi[P:], -Wi[:P], -Wi[P:]]
        w2s[f"w2_{w}"] = np.ascontiguousarray(
            np.stack(slabs, axis=1).astype(BF16))       # [P, nslab, OUT]

    in_maps = []
    b1 = B1.astype(BF16)
    b3 = B3.astype(BF16)
    for c in range(N_CORES):
        f = field_feat[c].transpose(1, 2, 0)                  # [i, x, bp]
        fx = f.reshape(NIG, 8, K, NUM_PART).transpose(0, 2, 1, 3)
        fx = fx.reshape(NIG // 4, 4, P, NUM_PART).transpose(0, 2, 1, 3)
        fx = np.ascontiguousarray(fx.astype(BF16))
        in_maps.append({"fieldx": fx, "b1": b1, "b3": b3, **w2s})
    return in_maps


def kernel(field_feat, weights):
    nc = _build()
    in_maps = _prep_inputs(field_feat, weights)
    trace = bool(int(os.environ.get("KERNEL_TRACE", "0")))
    # NRT occasionally reports a transient EXEC_UNIT_UNRECOVERABLE on the
    # first execute of a fresh session; a retry on a new session passes.
    for attempt in range(3):
        try:
            res = run_bass_kernel_spmd(nc, in_maps, list(range(N_CORES)),
                                       trace=trace)
            break
        except Exception:  # noqa: BLE001
            if attempt == 2:
                raise
    if trace:
        kernel.last_exec_time_ns = res.exec_time_ns
        kernel.last_results = res
    # out[jg, j8*16+y, bp] -> [bp, j, y]
    outs = []
    for c in range(N_CORES):
        o = np.asarray(res.results[c]["out"]).reshape(NJG, 8, K, NUM_PART)
        outs.append(o.transpose(3, 0, 1, 2).reshape(NUM_PART, OUT_FEAT, K))
    return np.stack(outs).reshape(BATCH, NUM_PART, OUT_FEAT, K).astype(np.float32)
